# revision 63
# baseline (speedup 1.0000x reference)
"""Trainium2 Bass kernel for nn_AttentionLayer (B=4, S=2048, H=16, DH=64).

Sharding: 8 cores = 4 batches x 2 head-halves. Core c handles batch c//2,
heads (c%2)*8 .. (c%2)*8+8 (512 of the 1024 QKV columns).

Design (v2, all three engines balanced at ~195us busy; measured 1.12e-2
rel err on HW, modeled ~237us vs the 287us bf16/ACT-only baseline):
  - Projections (PE, bf16): x[2048,1024] @ W[1024,512] per head-pair chunk.
    Q/K write TRANSPOSED via DVE epilogues straight from PSUM into fp8.
  - Scores in fp8e4m3 with DoubleRow (0.5 cycles/row, halves score PE time):
    kt8[m] [128p=2 heads x 64dh, S] fp8 holds k8; qt8[m] [128, 2, S] holds
    (q8, qres8 = q - q8). The DoubleRow matmul contracts
    k8^T q8 + k8^T qres8 = k8^T q  -- the q-side fp8 quantization error is
    residual-corrected for free using the second k-tile slot (the k8 operand
    is a stride-0 broadcast AP, verified on HW). Only the k-side fp8 error
    (~2.5% elementwise) survives -> ~1% on probs.
  - exp is split between ACT (~62%) and DVE (~38%): ACT slots run
    activation(Exp, bf16 out); DVE slots run ONE tensor_scalar
    t = s*A + B with int16 output, A = 128*log2(e)/8,
    B = 16256 - 128*0.0573. The round-to-nearest convert makes the int16
    bit pattern EXACTLY the Schraudolph bf16 approximation of exp(s/8)
    (verified bit-exact on HW); PV reads the tile bitcast as bf16.
    ~1.7% RMS elementwise on the offloaded fraction only.
  - Score tiles are PER-SLOT ([128,512], tag sc, bufs=4 = 4 banks): four
    independent exp pipeline chains, so the sc-buffer round trip
    (exp -> score matmul -> exp, ~500ns) stays off the critical path on
    both exp engines.
  - ctx accumulates per (head, qb) group in PSUM (tag ctx, bufs=2), zeroed
    by a PE matmul of zeros (start=True stop=True): a clean accumulation
    group enforces the WAR dependency on the previous group's norm reads
    (plain start=True on the skip_group_check PVs races it -> NaN), and it
    keeps zeroing off the DVE critical path. V tiles carry a ones-column so
    PV accumulates the softmax denominator in ctx col 64; norm = DVE
    reciprocal + broadcast mul, one DMA per group (split in halves for the
    final group to shorten the tail).
  - PSUM banks: qkv 2 + sc 4 + ctx 2 = 8.
  - Group order: pairs 0,1 h-major; pairs 2/3 interleaved at group level so
    the exp-heavy tail is shared by two pairs, with pair-3 ramping in late
    enough that its K/Q/V projections run on the merged phase's PE slack.
    K/Q units and V quads are explicitly scheduled at slot positions
    (kq_sched) so the qkv psum rotation matches execution order.
  - Startup: DMA order wk0,wq0,x0(4),wv0,x1(2),x2(2),wv1,x3(2),wv2,wv3,
    wk1,wq1 (wv in per-pair chunks from a [4,128,1024] host layout). ~13
    throwaway fp32 matmuls ramp the PE p-state during the DMA window; the
    (m0,c0) K/Q projections chase x0 sub-chunk arrivals j-interleaved.
"""

import numpy as np

B, S, H, DH = 4, 2048, 16, 64
D = H * DH  # 1024
NCORES = 8
COLS = 512  # qkv columns per core (8 heads)
NKB = 16
EXP_SCALE = 0.125  # 1/sqrt(DH)
LOG2E = 1.4426950408889634
A_EXP = 128.0 * LOG2E * 0.125
B_EXP = 16256.0 - 128.0 * 0.0573  # bf16 exp bias + Schraudolph centering

_CACHE = {}


def _build():
    import concourse.mybir as mybir
    import concourse.tile as tile
    from concourse import bacc

    f32 = mybir.dt.float32
    bf16 = mybir.dt.bfloat16
    fp8 = mybir.dt.float8e4
    i16 = mybir.dt.int16
    Alu = mybir.AluOpType
    Exp = mybir.ActivationFunctionType.Exp
    DR = mybir.MatmulPerfMode.DoubleRow

    nc = bacc.Bacc(
        "TRN2",
        target_bir_lowering=False,
        debug=False,
        enable_asserts=False,
        num_devices=NCORES,
    )

    xT_d = nc.dram_tensor("xT", [128, 4, 8, 512], bf16, kind="ExternalInput").ap()
    wq_d = nc.dram_tensor("wq", [128, 4, 8, 128], bf16, kind="ExternalInput").ap()
    wk_d = nc.dram_tensor("wk", [128, 4, 8, 128], bf16, kind="ExternalInput").ap()
    wv_d = nc.dram_tensor("wv", [4, 128, 1024], bf16, kind="ExternalInput").ap()
    bqk_d = nc.dram_tensor("bqk", [128, 8], f32, kind="ExternalInput").ap()
    bv_d = nc.dram_tensor("bv", [COLS], f32, kind="ExternalInput").ap()
    out_d = nc.dram_tensor("out", [S, COLS], f32, kind="ExternalOutput").ap()

    with tile.TileContext(nc) as tc:
        with (
            tc.tile_pool(name="consts", bufs=1) as consts,
            tc.tile_pool(name="wpool", bufs=1) as wpool,
            tc.tile_pool(name="qkp", bufs=1) as qkp,
            tc.tile_pool(name="vpool", bufs=1) as vpool,
            tc.tile_pool(name="xpool", bufs=1) as xpool,
            tc.tile_pool(name="epool", bufs=1) as epool,
            tc.tile_pool(name="opool", bufs=1) as opool,
            tc.tile_pool(name="psum", bufs=1, space="PSUM") as psum,
        ):
            from contextlib import contextmanager

            base = tc.cur_priority + 50
            att_cur = [base]
            fill_cur = [base + 8000]

            @contextmanager
            def band(cursor):
                off = tc.cur_priority - cursor[0]
                with tc.high_priority(offset=off):
                    yield
                    cursor[0] = tc.cur_priority

            # ---- constants + PE p-state warm-up ----
            with band(att_cur):
                warm = consts.tile([1, 1], f32)
                nc.vector.memset(warm, 0.0)
                nc.scalar.activation(warm, warm, Exp)  # pull ACT table load
                wsrc = consts.tile([128, 128], f32, name="wsrc")
                nc.vector.memset(wsrc, 0.0)
                zsrc = consts.tile([128, 260], mybir.dt.bfloat16, name="zsrc")
                nc.vector.memset(zsrc, 0.0)
                # single persistent 4-bank score tile; slots rotate regions
                # manually (subtile deps track the RAW/WAR ordering). ACT
                # exp reads region PAIRS [0:2] in one instruction.
                sc_all = psum.tile([128, 4, 512], f32, tag="sc", bufs=1,
                                   name="sc_all")
                for i in range(13):
                    nc.tensor.matmul(
                        sc_all[:, i % 4, 0:64],
                        lhsT=wsrc,
                        rhs=wsrc[:, 0:64],
                        start=True,
                        stop=True,
                    )

            with band(fill_cur):
                bqk_t = consts.tile([128, 8], f32)
                bv_s = consts.tile([1, COLS], f32)
                bvb = consts.tile([128, COLS], f32)
                nc.gpsimd.dma_start(out=bqk_t, in_=bqk_d)
                bq_t = bqk_t[:, 0:4]
                bk_t = bqk_t[:, 4:8]
                nc.gpsimd.dma_start(out=bv_s, in_=bv_d[None, :])
                nc.gpsimd.partition_broadcast(bvb, bv_s)

                vt = vpool.tile([128, NKB, 8, 65], bf16, name="vt")
                nc.vector.memset(vt[:, :, :, 64:65], 1.0)

                # fp8 q/k tiles: kt8[m] [128p = 2 heads x 64 dh, S];
                # qt8[m] [128, 2, S] = (q8, qres8)
                kt8 = [qkp.tile([128, S], fp8, name=f"kt8{m}") for m in range(4)]
                qt8 = [qkp.tile([128, 2, S], fp8, name=f"qt8{m}") for m in range(4)]
                wvt = [wpool.tile([128, 8, 128], bf16, name=f"wvt{m}") for m in range(4)]

            # ---- weight chunk ring ----
            wcur = {"q": {}, "k": {}}

            def load_w(proj, m):
                w_d = wq_d if proj == "q" else wk_d
                wt = wpool.tile(
                    [128, 8, 128], bf16, tag=f"w{proj}", bufs=2, name=f"w{proj}{m}"
                )
                nc.sync.dma_start(out=wt, in_=w_d[:, m, :, :])
                wcur[proj][m] = wt

            def load_wv(m):
                nc.sync.dma_start(
                    out=wvt[m], in_=wv_d[m].rearrange("p (j c) -> p j c", j=8)
                )

            with band(fill_cur):
                xt = [
                    xpool.tile([128, 8, 512], bf16, name=f"xt{c}") for c in range(4)
                ]

                # DMA order: startup-critical first; x0 split fine for
                # matmul chasing, later chunks coarser.
                load_w("k", 0)
                load_w("q", 0)
                for j0 in range(0, 8, 2):
                    nc.sync.dma_start(
                        out=xt[0][:, j0 : j0 + 2, :], in_=xT_d[:, 0, j0 : j0 + 2, :]
                    )
                load_wv(0)
                for j0 in range(0, 8, 4):
                    nc.sync.dma_start(
                        out=xt[1][:, j0 : j0 + 4, :], in_=xT_d[:, 1, j0 : j0 + 4, :]
                    )
                for j0 in range(0, 8, 4):
                    nc.sync.dma_start(
                        out=xt[2][:, j0 : j0 + 4, :], in_=xT_d[:, 2, j0 : j0 + 4, :]
                    )
                load_wv(1)
                for j0 in range(0, 8, 4):
                    nc.sync.dma_start(
                        out=xt[3][:, j0 : j0 + 4, :], in_=xT_d[:, 3, j0 : j0 + 4, :]
                    )
                load_wv(2)
                load_wv(3)
                load_w("k", 1)
                load_w("q", 1)

            # ---- projection epilogues (DVE, PSUM -> fp8 SBUF) ----
            def epi_k(m, c, ps):
                ch = slice(c * 512, (c + 1) * 512)
                with band(att_cur):
                    nc.vector.tensor_scalar_add(
                        kt8[m][:, ch], ps, bk_t[:, m : m + 1])

            def epi_q(m, c, ps):
                ch = slice(c * 512, (c + 1) * 512)
                with band(att_cur):
                    nc.vector.tensor_scalar_add(
                        qt8[m][:, 0, ch], ps, bq_t[:, m : m + 1])
                    nc.vector.scalar_tensor_tensor(
                        qt8[m][:, 1, ch],
                        ps,
                        bq_t[:, m : m + 1],
                        qt8[m][:, 0, ch],
                        Alu.add,
                        Alu.subtract,
                    )

            # ---- projection unit emitters (fill band) ----
            # Units are split into sub-chain parts emitted a couple of slots
            # apart so long PE chains do not clump ahead of the score
            # matmuls in PE program order.
            proj_state = {}

            def proj_kq_fused(m, c, part=None):
                key = ("kq", m, c)
                parts = range(4) if part is None else [part]
                for p in parts:
                    if p == 0:
                        proj_state[key] = (
                            psum.tile([128, 512], f32, tag="qkv", bufs=2,
                                      name="psk"),
                            psum.tile([128, 512], f32, tag="qkv", bufs=2,
                                      name="psq2"),
                        )
                    psk, psq = proj_state[key]
                    for j in (2 * p, 2 * p + 1):
                        nc.tensor.matmul(
                            psk, lhsT=wcur["k"][m][:, j, :], rhs=xt[c][:, j, :],
                            start=(j == 0), stop=(j == 7),
                        )
                        nc.tensor.matmul(
                            psq, lhsT=wcur["q"][m][:, j, :], rhs=xt[c][:, j, :],
                            start=(j == 0), stop=(j == 7),
                        )
                    if p == 3:
                        epi_k(m, c, psk)
                        epi_q(m, c, psq)

            def proj_qk(proj, m, c, part):
                key = (proj, m, c)
                if part == 0:
                    proj_state[key] = psum.tile(
                        [128, 512], f32, tag="qkv", bufs=2, name="psq"
                    )
                ps = proj_state[key]
                w = wcur[proj][m]
                for j in (range(4) if part == 0 else range(4, 8)):
                    nc.tensor.matmul(
                        ps,
                        lhsT=w[:, j, :],
                        rhs=xt[c][:, j, :],
                        start=(j == 0),
                        stop=(j == 7),
                    )
                if part == 1:
                    (epi_q if proj == "q" else epi_k)(m, c, ps)

            def proj_v(m, c, part=None):
                # 4 seq-subchunks of chunk c in one psum tile + one batched
                # DVE add into the vt tile; one sub-chain per part
                key = ("v", m, c)
                parts = range(4) if part is None else [part]
                for i in parts:
                    if i == 0:
                        proj_state[key] = psum.tile(
                            [128, 512], f32, tag="qkv", bufs=2, name="psv"
                        )
                    ps = proj_state[key]
                    for j in range(8):
                        nc.tensor.matmul(
                            ps[:, i * 128 : (i + 1) * 128],
                            lhsT=xt[c][:, j, i * 128 : (i + 1) * 128],
                            rhs=wvt[m][:, j, :],
                            # one start per bank-life: later sub-chunks'
                            # first writes consume the pending-zero bytes
                            start=(i == 0 and j == 0),
                            stop=(i == 3 and j == 7),
                            skip_group_check=True,
                        )
                    if i == 3:
                        with band(att_cur):
                            emit_vadd(m, c, ps)

            def emit_vadd(m, c, ps):
                        nc.vector.tensor_add(
                            vt[:, 4 * c : 4 * c + 4, 2 * m : 2 * m + 2, 0:64],
                            ps.rearrange("p (i h d) -> p i h d", i=4, h=2),
                            bvb[:, m * 128 : (m + 1) * 128]
                            .rearrange("p (one h d) -> p one h d", one=1, h=2)
                            .broadcast_to([128, 4, 2, 64]),
                        )

            # ---- attention stream ----
            # pair-0/1 groups h-major; pairs 2 and 3 interleave at group
            # granularity so the exp-heavy tail is shared by both pairs
            # (pair-3 slots start right after its projections land). The
            # first two groups (h0/h1 at qb0) interleave per k-block (each
            # gets its own ctx bank via bufs=2).
            groups = [(h, qb) for h in (0, 1, 2, 3) for qb in range(4)]
            # pair-2 first, then pairs 2/3 interleaved: pair-3's first group
            # starts at slot 336, so ALL of pair-3's K/Q/V projection work
            # lands in the merged phase's PE-slack zone (t>~150)
            groups += [(4, 0), (4, 1), (4, 2), (4, 3), (5, 0), (6, 0),
                       (5, 1), (6, 1), (5, 2), (6, 2), (5, 3), (6, 3),
                       (7, 0), (7, 1), (7, 2), (7, 3)]
            slots = []
            for kb in range(NKB):
                slots.append((0, 0, kb))
                slots.append((1, 0, kb))
            slots += [
                (h, qb, kb)
                for (h, qb) in groups
                if (h, qb) not in ((0, 0), (1, 0))
                for kb in range(NKB)
            ]
            NS = len(slots)

            # engine assignment: 3/8 DVE in the proj-heavy first half,
            # 7/16 in the balanced second half; none in the startup slots
            # (DVE is busy with the critical projection epilogues there).
            # DVE slots sit at s%4 in {2,3} so that ACT slots form
            # region-aligned pairs [0:2] sharing one exp instruction.
            def is_dve(s):
                if s < 24:
                    return False
                if s >= NS - 3:
                    return s % 2 == 1
                if s < 256:
                    return s % 8 in (2, 3, 7)
                return s % 16 in (2, 3, 6, 7, 10, 11, 14)

            def pair_start(s):
                return (
                    s % 4 == 0
                    and s + 1 < NS
                    and not is_dve(s)
                    and not is_dve(s + 1)
                )

            # All projection units explicitly scheduled at slot positions so
            # the qkv psum rotation matches execution order. V quads for
            # pair m land shortly before pair-m attention consumes them.
            kq_sched = {}

            def sched(s, item):
                kq_sched.setdefault(s, []).append(item)

            PSPC = 0  # sub-chain parts of one unit are emitted together

            def sched_qk(s, proj, m, c):
                sched(s, (proj, m, c, 0))
                sched(s + PSPC, (proj, m, c, 1))

            def sched_v(s, m, c):
                for i in range(4):
                    sched(s + PSPC * i, ("v", m, c, i))

            # pair 0: k chunks feed the kb-consuming prefix (early); q chunks
            # are only needed per qb-group start (32/48/64) so they spread
            for c in range(1, 4):
                sched_qk(4 * c - 2, "k", 0, c)
                sched_qk(8 + 16 * (c - 1), "q", 0, c)
            for c in range(1, 4):  # v(0,0) is emitted in the prologue
                sched_v(8 * c - 6, 0, c)

            def sched_pair(m, kq0, ks, qs, vs):
                if m >= 2:  # pair-1 weights are loaded in the startup block
                    sched(kq0 - 6, ("wl", "k", m))
                    sched(kq0 - 4, ("wl", "q", m))
                for p in range(4):
                    sched(kq0 + PSPC * p, ("kq", m, 0, p))
                for c in range(1, 4):
                    sched_qk(ks[c - 1], "k", m, c)
                    sched_qk(qs[c - 1], "q", m, c)
                for c in range(4):
                    sched_v(vs + 4 * c, m, c)

            # pair 1 window starts slot 128; pair 2 at 256; pair 3 at 336
            sched_pair(1, 66, (72, 82, 92), (76, 86, 96), 104)
            sched_pair(2, 136, (146, 156, 166), (248, 268, 288), 224)
            sched_pair(3, 288, (296, 304, 312), (326, 356, 386), 316)

            # pair-3's projection units are emitted in the ATTENTION band:
            # a hard position in PE program order prevents the greedy
            # scheduler from deferring them until their consumers stall;
            # earlier pairs stay in the fill band (their windows have real
            # PE contention where floating helps)
            def emit_fill(s):
                for u in kq_sched.get(s, ()):
                    m = u[2] if u[0] in ("wl",) else u[1]
                    with band(att_cur if m == 3 else fill_cur):
                        if u[0] == "wl":
                            load_w(u[1], u[2])
                        elif u[0] == "kq":
                            proj_kq_fused(u[1], u[2], u[3])
                        elif u[0] == "v":
                            proj_v(u[1], u[2], u[3])
                        else:
                            proj_qk(u[0], u[1], u[2], u[3])

            def emit_scores(s):
                h, qb, kb = slots[s]
                m, p0 = h // 2, 64 * (h % 2)
                with band(att_cur):
                    kb_ap = (
                        kt8[m][p0 : p0 + 64, kb * 128 : (kb + 1) * 128]
                        .rearrange("p (one f) -> p one f", one=1)
                        .broadcast_to([64, 2, 128])
                    )
                    nc.tensor.matmul(
                        sc_all[:, s % 4, :],
                        lhsT=kb_ap,
                        rhs=qt8[m][p0 : p0 + 64, :, qb * 512 : (qb + 1) * 512],
                        start=True,
                        stop=True,
                        perf_mode=DR,
                    )

            def emit_exp(pend):
                # pend: 1 slot (single/DVE) or 2 ACT slots on regions [0:2]
                s0 = pend[0]
                with band(att_cur):
                    if len(pend) == 2:
                        ee = epool.tile([128, 2, 512], mybir.dt.bfloat16,
                                        tag="ep", bufs=4, name="ep")
                        nc.scalar.activation(
                            ee, sc_all[:, 0:2, :], Exp, scale=EXP_SCALE
                        )
                        return [ee[:, 0, :], ee[:, 1, :]]
                    if is_dve(s0):
                        ed = epool.tile([128, 512], i16, tag="ed", bufs=6,
                                        name="ed")
                        nc.vector.tensor_scalar(
                            ed, sc_all[:, s0 % 4, :], A_EXP, B_EXP,
                            Alu.mult, Alu.add,
                        )
                        return [ed.bitcast(mybir.dt.bfloat16)]
                    ee = epool.tile([128, 512], mybir.dt.bfloat16, tag="ee",
                                    bufs=6, name="ee")
                    nc.scalar.activation(
                        ee, sc_all[:, s0 % 4, :], Exp, scale=EXP_SCALE
                    )
                    return [ee]

            ctx_of = {}

            def do_pv(ev, h, qb, kb):
                if kb == 0:
                    ctx_of[(h, qb)] = psum.tile(
                        [128, 4, 65], f32, tag="ctx", bufs=2, name="ctx"
                    )
                    # zero via a PE matmul: a clean start/stop group, so the
                    # WAR dep on the previous group's norm reads is enforced
                    # (skip_group_check PVs alone race it), and it keeps the
                    # zeroing off the DVE critical path.
                    nc.tensor.matmul(
                        ctx_of[(h, qb)].rearrange("p t d -> p (t d)"),
                        lhsT=zsrc[:, 0:128],
                        rhs=zsrc,
                        start=True,
                        stop=True,
                    )
                ctx = ctx_of[(h, qb)]
                for qt in range(4):
                    nc.tensor.matmul(
                        ctx[:, qt, :],
                        lhsT=ev[:, qt * 128 : (qt + 1) * 128],
                        rhs=vt[:, kb, h, :],
                        start=False,
                        stop=False,
                        skip_group_check=True,
                    )

            def do_norm(h, qb, t0, t1):
                ctx = ctx_of[(h, qb)]
                nt = t1 - t0
                rr = opool.tile([128, 4, 1], f32, tag="r", bufs=3, name="rr")
                nc.vector.reciprocal(rr[:, t0:t1], ctx[:, t0:t1, 64:65])
                ob = opool.tile([128, 4, 64], f32, tag="o", bufs=3, name="ob")
                nc.vector.tensor_mul(
                    ob[:, t0:t1], ctx[:, t0:t1, 0:64],
                    rr[:, t0:t1].broadcast_to([128, nt, 64]),
                )
                nc.sync.dma_start(
                    out=out_d[
                        qb * 512 + t0 * 128 : qb * 512 + t1 * 128,
                        h * 64 : (h + 1) * 64,
                    ].rearrange("(t p) d -> p t d", p=128),
                    in_=ob[:, t0:t1],
                )

            last_group = groups[-1]

            def emit_pv(s, ev):
                h, qb, kb = slots[s]
                with band(att_cur):
                    if kb == NKB - 1 and (h, qb) == last_group:
                        # split the final norm to shorten the tail
                        ctx = ctx_of[(h, qb)]
                        for qt in range(4):
                            nc.tensor.matmul(
                                ctx[:, qt, :],
                                lhsT=ev[:, qt * 128 : (qt + 1) * 128],
                                rhs=vt[:, kb, h, :],
                                start=False,
                                stop=False,
                                skip_group_check=True,
                            )
                            if qt == 1:
                                do_norm(h, qb, 0, 2)
                        do_norm(h, qb, 2, 4)
                        return
                    do_pv(ev, h, qb, kb)
                    if kb == NKB - 1:
                        do_norm(h, qb, 0, 4)

            with band(fill_cur):
                proj_kq_fused(0, 0)
            for s in range(4):
                emit_scores(s)
            with band(fill_cur):
                proj_v(0, 0)
            pend = []
            for s in range(NS):
                pend.append(s)
                if pair_start(s):
                    emit_fill(s)
                    continue
                evs = emit_exp(pend)
                for t, ev in zip(pend, evs):
                    emit_pv(t, ev)
                # score region t%4 is free once the exp that read it has
                # been emitted; emission order defines the WAR dep
                for t in pend:
                    if t + 4 < NS:
                        emit_scores(t + 4)
                pend = []
                emit_fill(s)

    nc.compile()
    return nc


def _get_nc():
    if "nc" not in _CACHE:
        _CACHE["nc"] = _build()
    return _CACHE["nc"]


def _in_maps(x, Wq, bq, Wk, bk, Wv, bv):
    import ml_dtypes

    bf = ml_dtypes.bfloat16
    x = np.asarray(x, np.float32)
    maps = []
    for c in range(NCORES):
        b, hh = c // 2, c % 2
        cs = slice(hh * COLS, (hh + 1) * COLS)

        def warr(W):
            # [1024, 512] -> [128 p, 4 m, 8 j, 128 c]
            a = np.asarray(W, np.float32)[:, cs].astype(bf)
            return np.ascontiguousarray(
                a.reshape(8, 128, 4, 128).transpose(1, 2, 0, 3)
            )

        xTr = x[b].T.astype(bf).reshape(8, 128, 4, 512).transpose(1, 2, 0, 3)
        # wv: [1024, 512] -> [4 m, 128 p, 8 j * 128 c]
        wvr = np.asarray(Wv, np.float32)[:, cs].astype(bf)
        wvr = wvr.reshape(8, 128, 4, 128).transpose(2, 1, 0, 3).reshape(4, 128, 1024)
        maps.append(
            {
                "xT": np.ascontiguousarray(xTr),
                "wq": warr(Wq),
                "wk": warr(Wk),
                "wv": np.ascontiguousarray(wvr),
                "bqk": np.ascontiguousarray(
                    np.concatenate(
                        [
                            np.asarray(bq, np.float32)[cs].reshape(4, 128).T,
                            np.asarray(bk, np.float32)[cs].reshape(4, 128).T,
                        ],
                        axis=1,
                    )
                ),
                "bv": np.ascontiguousarray(np.asarray(bv, np.float32)[cs]),
            }
        )
    return maps


def _run(inputs, trace=False):
    from concourse import bass_utils

    nc = _get_nc()
    res = bass_utils.run_bass_kernel_spmd(
        nc,
        _in_maps(**inputs),
        core_ids=list(range(NCORES)),
        trace=trace,
    )
    out = np.empty((B, S, D), np.float32)
    for c in range(NCORES):
        b, hh = c // 2, c % 2
        out[b, :, hh * COLS : (hh + 1) * COLS] = res.results[c]["out"]
    return out, res


def kernel(**inputs):
    out, _ = _run(inputs, trace=False)
    return out


if __name__ == "__main__":
    _get_nc()
    print("build ok")


# revision 64
# speedup vs baseline: 1.0137x; 1.0137x over previous
"""Trainium2 Bass kernel for nn_AttentionLayer (B=4, S=2048, H=16, DH=64).

Sharding: 8 cores = 4 batches x 2 head-halves. Core c handles batch c//2,
heads (c%2)*8 .. (c%2)*8+8 (512 of the 1024 QKV columns).

Design (v2, all three engines balanced at ~195us busy; measured 1.12e-2
rel err on HW, modeled ~237us vs the 287us bf16/ACT-only baseline):
  - Projections (PE, bf16): x[2048,1024] @ W[1024,512] per head-pair chunk.
    Q/K write TRANSPOSED via DVE epilogues straight from PSUM into fp8.
  - Scores in fp8e4m3 with DoubleRow (0.5 cycles/row, halves score PE time):
    kt8[m] [128p=2 heads x 64dh, S] fp8 holds k8; qt8[m] [128, 2, S] holds
    (q8, qres8 = q - q8). The DoubleRow matmul contracts
    k8^T q8 + k8^T qres8 = k8^T q  -- the q-side fp8 quantization error is
    residual-corrected for free using the second k-tile slot (the k8 operand
    is a stride-0 broadcast AP, verified on HW). Only the k-side fp8 error
    (~2.5% elementwise) survives -> ~1% on probs.
  - exp is split between ACT (~62%) and DVE (~38%): ACT slots run
    activation(Exp, bf16 out); DVE slots run ONE tensor_scalar
    t = s*A + B with int16 output, A = 128*log2(e)/8,
    B = 16256 - 128*0.0573. The round-to-nearest convert makes the int16
    bit pattern EXACTLY the Schraudolph bf16 approximation of exp(s/8)
    (verified bit-exact on HW); PV reads the tile bitcast as bf16.
    ~1.7% RMS elementwise on the offloaded fraction only.
  - Score tiles are PER-SLOT ([128,512], tag sc, bufs=4 = 4 banks): four
    independent exp pipeline chains, so the sc-buffer round trip
    (exp -> score matmul -> exp, ~500ns) stays off the critical path on
    both exp engines.
  - ctx accumulates per (head, qb) group in PSUM (tag ctx, bufs=2), zeroed
    by a PE matmul of zeros (start=True stop=True): a clean accumulation
    group enforces the WAR dependency on the previous group's norm reads
    (plain start=True on the skip_group_check PVs races it -> NaN), and it
    keeps zeroing off the DVE critical path. V tiles carry a ones-column so
    PV accumulates the softmax denominator in ctx col 64; norm = DVE
    reciprocal + broadcast mul, one DMA per group (split in halves for the
    final group to shorten the tail).
  - PSUM banks: qkv 2 + sc 4 + ctx 2 = 8.
  - Group order: pairs 0,1 h-major; pairs 2/3 interleaved at group level so
    the exp-heavy tail is shared by two pairs, with pair-3 ramping in late
    enough that its K/Q/V projections run on the merged phase's PE slack.
    K/Q units and V quads are explicitly scheduled at slot positions
    (kq_sched) so the qkv psum rotation matches execution order.
  - Startup: DMA order wk0,wq0,x0(4),wv0,x1(2),x2(2),wv1,x3(2),wv2,wv3,
    wk1,wq1 (wv in per-pair chunks from a [4,128,1024] host layout). ~13
    throwaway fp32 matmuls ramp the PE p-state during the DMA window; the
    (m0,c0) K/Q projections chase x0 sub-chunk arrivals j-interleaved.
"""

import numpy as np

B, S, H, DH = 4, 2048, 16, 64
D = H * DH  # 1024
NCORES = 8
COLS = 512  # qkv columns per core (8 heads)
NKB = 16
EXP_SCALE = 0.125  # 1/sqrt(DH)
LOG2E = 1.4426950408889634
A_EXP = 128.0 * LOG2E * 0.125
B_EXP = 16256.0 - 128.0 * 0.0573  # bf16 exp bias + Schraudolph centering

_CACHE = {}


def _build():
    import concourse.mybir as mybir
    import concourse.tile as tile
    from concourse import bacc

    f32 = mybir.dt.float32
    bf16 = mybir.dt.bfloat16
    fp8 = mybir.dt.float8e4
    i16 = mybir.dt.int16
    Alu = mybir.AluOpType
    Exp = mybir.ActivationFunctionType.Exp
    DR = mybir.MatmulPerfMode.DoubleRow

    nc = bacc.Bacc(
        "TRN2",
        target_bir_lowering=False,
        debug=False,
        enable_asserts=False,
        num_devices=NCORES,
    )

    xT_d = nc.dram_tensor("xT", [128, 4, 8, 512], bf16, kind="ExternalInput").ap()
    wq_d = nc.dram_tensor("wq", [128, 4, 8, 128], bf16, kind="ExternalInput").ap()
    wk_d = nc.dram_tensor("wk", [128, 4, 8, 128], bf16, kind="ExternalInput").ap()
    wv_d = nc.dram_tensor("wv", [4, 128, 1024], bf16, kind="ExternalInput").ap()
    bqk_d = nc.dram_tensor("bqk", [128, 8], f32, kind="ExternalInput").ap()
    bv_d = nc.dram_tensor("bv", [COLS], f32, kind="ExternalInput").ap()
    out_d = nc.dram_tensor("out", [S, COLS], f32, kind="ExternalOutput").ap()

    with tile.TileContext(nc) as tc:
        with (
            tc.tile_pool(name="consts", bufs=1) as consts,
            tc.tile_pool(name="wpool", bufs=1) as wpool,
            tc.tile_pool(name="qkp", bufs=1) as qkp,
            tc.tile_pool(name="vpool", bufs=1) as vpool,
            tc.tile_pool(name="xpool", bufs=1) as xpool,
            tc.tile_pool(name="epool", bufs=1) as epool,
            tc.tile_pool(name="opool", bufs=1) as opool,
            tc.tile_pool(name="psum", bufs=1, space="PSUM") as psum,
        ):
            from contextlib import contextmanager

            base = tc.cur_priority + 50
            att_cur = [base]
            fill_cur = [base + 8000]

            @contextmanager
            def band(cursor):
                off = tc.cur_priority - cursor[0]
                with tc.high_priority(offset=off):
                    yield
                    cursor[0] = tc.cur_priority

            # ---- constants + PE p-state warm-up ----
            with band(att_cur):
                warm = consts.tile([1, 1], f32)
                nc.vector.memset(warm, 0.0)
                nc.scalar.activation(warm, warm, Exp)  # pull ACT table load
                wsrc = consts.tile([128, 128], f32, name="wsrc")
                nc.vector.memset(wsrc, 0.0)
                zsrc = consts.tile([128, 260], mybir.dt.bfloat16, name="zsrc")
                nc.vector.memset(zsrc, 0.0)
                # single persistent 4-bank score tile; slots rotate regions
                # manually (subtile deps track the RAW/WAR ordering). ACT
                # exp reads region PAIRS [0:2] in one instruction.
                sc_all = psum.tile([128, 4, 512], f32, tag="sc", bufs=1,
                                   name="sc_all")
                for i in range(13):
                    nc.tensor.matmul(
                        sc_all[:, i % 4, 0:64],
                        lhsT=wsrc,
                        rhs=wsrc[:, 0:64],
                        start=True,
                        stop=True,
                    )

            with band(fill_cur):
                bqk_t = consts.tile([128, 8], f32)
                bv_s = consts.tile([1, COLS], f32)
                bvb = consts.tile([128, COLS], f32)
                nc.gpsimd.dma_start(out=bqk_t, in_=bqk_d)
                bq_t = bqk_t[:, 0:4]
                bk_t = bqk_t[:, 4:8]
                nc.gpsimd.dma_start(out=bv_s, in_=bv_d[None, :])
                nc.gpsimd.partition_broadcast(bvb, bv_s)

                vt = vpool.tile([128, NKB, 8, 65], bf16, name="vt")
                nc.vector.memset(vt[:, :, :, 64:65], 1.0)

                # fp8 q/k tiles: kt8[m] [128p = 2 heads x 64 dh, S];
                # qt8[m] [128, 2, S] = (q8, qres8)
                kt8 = [qkp.tile([128, S], fp8, name=f"kt8{m}") for m in range(4)]
                qt8 = [qkp.tile([128, 2, S], fp8, name=f"qt8{m}") for m in range(4)]
                wvt = [wpool.tile([128, 8, 128], bf16, name=f"wvt{m}") for m in range(4)]

            # ---- weight chunk ring ----
            wcur = {"q": {}, "k": {}}

            def load_w(proj, m):
                w_d = wq_d if proj == "q" else wk_d
                wt = wpool.tile(
                    [128, 8, 128], bf16, tag=f"w{proj}", bufs=2, name=f"w{proj}{m}"
                )
                nc.sync.dma_start(out=wt, in_=w_d[:, m, :, :])
                wcur[proj][m] = wt

            def load_wv(m):
                nc.sync.dma_start(
                    out=wvt[m], in_=wv_d[m].rearrange("p (j c) -> p j c", j=8)
                )

            with band(fill_cur):
                xt = [
                    xpool.tile([128, 8, 512], bf16, name=f"xt{c}") for c in range(4)
                ]

                # DMA order: startup-critical first; x0 split fine for
                # matmul chasing, later chunks coarser.
                load_w("k", 0)
                load_w("q", 0)
                for j0 in range(0, 8, 2):
                    nc.sync.dma_start(
                        out=xt[0][:, j0 : j0 + 2, :], in_=xT_d[:, 0, j0 : j0 + 2, :]
                    )
                load_wv(0)
                for j0 in range(0, 8, 4):
                    nc.sync.dma_start(
                        out=xt[1][:, j0 : j0 + 4, :], in_=xT_d[:, 1, j0 : j0 + 4, :]
                    )
                for j0 in range(0, 8, 4):
                    nc.sync.dma_start(
                        out=xt[2][:, j0 : j0 + 4, :], in_=xT_d[:, 2, j0 : j0 + 4, :]
                    )
                for j0 in range(0, 8, 4):
                    nc.sync.dma_start(
                        out=xt[3][:, j0 : j0 + 4, :], in_=xT_d[:, 3, j0 : j0 + 4, :]
                    )
                load_wv(1)
                load_wv(2)
                load_wv(3)
                load_w("k", 1)
                load_w("q", 1)

            # ---- projection epilogues (DVE, PSUM -> fp8 SBUF) ----
            def epi_k(m, c, ps):
                ch = slice(c * 512, (c + 1) * 512)
                with band(att_cur):
                    nc.vector.tensor_scalar_add(
                        kt8[m][:, ch], ps, bk_t[:, m : m + 1])

            def epi_q(m, c, ps):
                ch = slice(c * 512, (c + 1) * 512)
                with band(att_cur):
                    nc.vector.tensor_scalar_add(
                        qt8[m][:, 0, ch], ps, bq_t[:, m : m + 1])
                    nc.vector.scalar_tensor_tensor(
                        qt8[m][:, 1, ch],
                        ps,
                        bq_t[:, m : m + 1],
                        qt8[m][:, 0, ch],
                        Alu.add,
                        Alu.subtract,
                    )

            # ---- projection unit emitters (fill band) ----
            # Units are split into sub-chain parts emitted a couple of slots
            # apart so long PE chains do not clump ahead of the score
            # matmuls in PE program order.
            proj_state = {}

            def proj_kq_fused(m, c, part=None):
                key = ("kq", m, c)
                parts = range(4) if part is None else [part]
                for p in parts:
                    if p == 0:
                        proj_state[key] = (
                            psum.tile([128, 512], f32, tag="qkv", bufs=2,
                                      name="psk"),
                            psum.tile([128, 512], f32, tag="qkv", bufs=2,
                                      name="psq2"),
                        )
                    psk, psq = proj_state[key]
                    for j in (2 * p, 2 * p + 1):
                        nc.tensor.matmul(
                            psk, lhsT=wcur["k"][m][:, j, :], rhs=xt[c][:, j, :],
                            start=(j == 0), stop=(j == 7),
                        )
                        nc.tensor.matmul(
                            psq, lhsT=wcur["q"][m][:, j, :], rhs=xt[c][:, j, :],
                            start=(j == 0), stop=(j == 7),
                        )
                    if p == 3:
                        epi_k(m, c, psk)
                        epi_q(m, c, psq)

            def proj_qk(proj, m, c, part):
                key = (proj, m, c)
                if part == 0:
                    proj_state[key] = psum.tile(
                        [128, 512], f32, tag="qkv", bufs=2, name="psq"
                    )
                ps = proj_state[key]
                w = wcur[proj][m]
                for j in (range(4) if part == 0 else range(4, 8)):
                    nc.tensor.matmul(
                        ps,
                        lhsT=w[:, j, :],
                        rhs=xt[c][:, j, :],
                        start=(j == 0),
                        stop=(j == 7),
                    )
                if part == 1:
                    (epi_q if proj == "q" else epi_k)(m, c, ps)

            def proj_v(m, c, part=None):
                # 4 seq-subchunks of chunk c in one psum tile + one batched
                # DVE add into the vt tile; one sub-chain per part
                key = ("v", m, c)
                parts = range(4) if part is None else [part]
                for i in parts:
                    if i == 0:
                        proj_state[key] = psum.tile(
                            [128, 512], f32, tag="qkv", bufs=2, name="psv"
                        )
                    ps = proj_state[key]
                    for j in range(8):
                        nc.tensor.matmul(
                            ps[:, i * 128 : (i + 1) * 128],
                            lhsT=xt[c][:, j, i * 128 : (i + 1) * 128],
                            rhs=wvt[m][:, j, :],
                            # one start per bank-life: later sub-chunks'
                            # first writes consume the pending-zero bytes
                            start=(i == 0 and j == 0),
                            stop=(i == 3 and j == 7),
                            skip_group_check=True,
                        )
                    if i == 3:
                        with band(att_cur):
                            emit_vadd(m, c, ps)

            def emit_vadd(m, c, ps):
                        nc.vector.tensor_add(
                            vt[:, 4 * c : 4 * c + 4, 2 * m : 2 * m + 2, 0:64],
                            ps.rearrange("p (i h d) -> p i h d", i=4, h=2),
                            bvb[:, m * 128 : (m + 1) * 128]
                            .rearrange("p (one h d) -> p one h d", one=1, h=2)
                            .broadcast_to([128, 4, 2, 64]),
                        )

            # ---- attention stream ----
            # pair-0/1 groups h-major; pairs 2 and 3 interleave at group
            # granularity so the exp-heavy tail is shared by both pairs
            # (pair-3 slots start right after its projections land). The
            # first two groups (h0/h1 at qb0) interleave per k-block (each
            # gets its own ctx bank via bufs=2).
            groups = [(h, qb) for h in (0, 1, 2, 3) for qb in range(4)]
            # pair-2 first, then pairs 2/3 interleaved: pair-3's first group
            # starts at slot 336, so ALL of pair-3's K/Q/V projection work
            # lands in the merged phase's PE-slack zone (t>~150)
            groups += [(4, 0), (4, 1), (4, 2), (4, 3), (5, 0), (6, 0),
                       (5, 1), (6, 1), (5, 2), (6, 2), (5, 3), (6, 3),
                       (7, 0), (7, 1), (7, 2), (7, 3)]
            slots = []
            for kb in range(NKB):
                slots.append((0, 0, kb))
                slots.append((1, 0, kb))
            slots += [
                (h, qb, kb)
                for (h, qb) in groups
                if (h, qb) not in ((0, 0), (1, 0))
                for kb in range(NKB)
            ]
            NS = len(slots)

            # engine assignment: 3/8 DVE in the proj-heavy first half,
            # 7/16 in the balanced second half; none in the startup slots
            # (DVE is busy with the critical projection epilogues there).
            # DVE slots sit at s%4 in {2,3} so that ACT slots form
            # region-aligned pairs [0:2] sharing one exp instruction.
            def is_dve(s):
                if s < 24:
                    return False
                if s >= NS - 3:
                    return s % 2 == 1
                if s < 256:
                    return s % 8 in (2, 3, 7)
                return s % 16 in (2, 3, 6, 7, 10, 11, 14)

            def pair_start(s):
                return (
                    s % 4 == 0
                    and s + 1 < NS
                    and not is_dve(s)
                    and not is_dve(s + 1)
                )

            # All projection units explicitly scheduled at slot positions so
            # the qkv psum rotation matches execution order. V quads for
            # pair m land shortly before pair-m attention consumes them.
            kq_sched = {}

            def sched(s, item):
                kq_sched.setdefault(s, []).append(item)

            PSPC = 0  # sub-chain parts of one unit are emitted together

            def sched_qk(s, proj, m, c):
                sched(s, (proj, m, c, 0))
                sched(s + PSPC, (proj, m, c, 1))

            def sched_v(s, m, c):
                for i in range(4):
                    sched(s + PSPC * i, ("v", m, c, i))

            # pair 0: k chunks feed the kb-consuming prefix (early); q chunks
            # are only needed per qb-group start (32/48/64) so they spread
            for c in range(1, 4):
                sched_qk(4 * c - 2, "k", 0, c)
                sched_qk(8 + 16 * (c - 1), "q", 0, c)
            for c in range(1, 4):  # v(0,0) is emitted in the prologue
                sched_v(8 * c - 6, 0, c)

            def sched_pair(m, kq0, ks, qs, vs):
                if m >= 2:  # pair-1 weights are loaded in the startup block
                    sched(kq0 - 6, ("wl", "k", m))
                    sched(kq0 - 4, ("wl", "q", m))
                for p in range(4):
                    sched(kq0 + PSPC * p, ("kq", m, 0, p))
                for c in range(1, 4):
                    sched_qk(ks[c - 1], "k", m, c)
                    sched_qk(qs[c - 1], "q", m, c)
                for c in range(4):
                    sched_v(vs + 4 * c, m, c)

            # pair 1 window starts slot 128; pair 2 at 256; pair 3 at 336
            sched_pair(1, 66, (72, 82, 92), (76, 86, 96), 104)
            sched_pair(2, 136, (146, 156, 166), (248, 268, 288), 224)
            sched_pair(3, 288, (296, 304, 312), (326, 356, 386), 316)

            # pair-3's projection units are emitted in the ATTENTION band:
            # a hard position in PE program order prevents the greedy
            # scheduler from deferring them until their consumers stall;
            # earlier pairs stay in the fill band (their windows have real
            # PE contention where floating helps)
            def emit_fill(s):
                for u in kq_sched.get(s, ()):
                    m = u[2] if u[0] in ("wl",) else u[1]
                    with band(att_cur if m == 3 else fill_cur):
                        if u[0] == "wl":
                            load_w(u[1], u[2])
                        elif u[0] == "kq":
                            proj_kq_fused(u[1], u[2], u[3])
                        elif u[0] == "v":
                            proj_v(u[1], u[2], u[3])
                        else:
                            proj_qk(u[0], u[1], u[2], u[3])

            def emit_scores(s):
                h, qb, kb = slots[s]
                m, p0 = h // 2, 64 * (h % 2)
                with band(att_cur):
                    kb_ap = (
                        kt8[m][p0 : p0 + 64, kb * 128 : (kb + 1) * 128]
                        .rearrange("p (one f) -> p one f", one=1)
                        .broadcast_to([64, 2, 128])
                    )
                    nc.tensor.matmul(
                        sc_all[:, s % 4, :],
                        lhsT=kb_ap,
                        rhs=qt8[m][p0 : p0 + 64, :, qb * 512 : (qb + 1) * 512],
                        start=True,
                        stop=True,
                        perf_mode=DR,
                    )

            def emit_exp(pend):
                # pend: 1 slot (single/DVE) or 2 ACT slots on regions [0:2]
                s0 = pend[0]
                with band(att_cur):
                    if len(pend) == 2:
                        ee = epool.tile([128, 2, 512], mybir.dt.bfloat16,
                                        tag="ep", bufs=4, name="ep")
                        nc.scalar.activation(
                            ee, sc_all[:, 0:2, :], Exp, scale=EXP_SCALE
                        )
                        return [ee[:, 0, :], ee[:, 1, :]]
                    if is_dve(s0):
                        ed = epool.tile([128, 512], i16, tag="ed", bufs=6,
                                        name="ed")
                        nc.vector.tensor_scalar(
                            ed, sc_all[:, s0 % 4, :], A_EXP, B_EXP,
                            Alu.mult, Alu.add,
                        )
                        return [ed.bitcast(mybir.dt.bfloat16)]
                    ee = epool.tile([128, 512], mybir.dt.bfloat16, tag="ee",
                                    bufs=6, name="ee")
                    nc.scalar.activation(
                        ee, sc_all[:, s0 % 4, :], Exp, scale=EXP_SCALE
                    )
                    return [ee]

            ctx_of = {}

            def do_pv(ev, h, qb, kb):
                if kb == 0:
                    ctx_of[(h, qb)] = psum.tile(
                        [128, 4, 65], f32, tag="ctx", bufs=2, name="ctx"
                    )
                    # zero via a PE matmul: a clean start/stop group, so the
                    # WAR dep on the previous group's norm reads is enforced
                    # (skip_group_check PVs alone race it), and it keeps the
                    # zeroing off the DVE critical path.
                    nc.tensor.matmul(
                        ctx_of[(h, qb)].rearrange("p t d -> p (t d)"),
                        lhsT=zsrc[:, 0:128],
                        rhs=zsrc,
                        start=True,
                        stop=True,
                    )
                ctx = ctx_of[(h, qb)]
                for qt in range(4):
                    nc.tensor.matmul(
                        ctx[:, qt, :],
                        lhsT=ev[:, qt * 128 : (qt + 1) * 128],
                        rhs=vt[:, kb, h, :],
                        start=False,
                        stop=False,
                        skip_group_check=True,
                    )

            def do_norm(h, qb, t0, t1):
                ctx = ctx_of[(h, qb)]
                nt = t1 - t0
                rr = opool.tile([128, 4, 1], f32, tag="r", bufs=3, name="rr")
                nc.vector.reciprocal(rr[:, t0:t1], ctx[:, t0:t1, 64:65])
                ob = opool.tile([128, 4, 64], f32, tag="o", bufs=3, name="ob")
                nc.vector.tensor_mul(
                    ob[:, t0:t1], ctx[:, t0:t1, 0:64],
                    rr[:, t0:t1].broadcast_to([128, nt, 64]),
                )
                nc.sync.dma_start(
                    out=out_d[
                        qb * 512 + t0 * 128 : qb * 512 + t1 * 128,
                        h * 64 : (h + 1) * 64,
                    ].rearrange("(t p) d -> p t d", p=128),
                    in_=ob[:, t0:t1],
                )

            last_group = groups[-1]

            def emit_pv(s, ev):
                h, qb, kb = slots[s]
                with band(att_cur):
                    if kb == NKB - 1 and (h, qb) == last_group:
                        # split the final norm to shorten the tail
                        ctx = ctx_of[(h, qb)]
                        for qt in range(4):
                            nc.tensor.matmul(
                                ctx[:, qt, :],
                                lhsT=ev[:, qt * 128 : (qt + 1) * 128],
                                rhs=vt[:, kb, h, :],
                                start=False,
                                stop=False,
                                skip_group_check=True,
                            )
                            if qt == 1:
                                do_norm(h, qb, 0, 2)
                        do_norm(h, qb, 2, 4)
                        return
                    do_pv(ev, h, qb, kb)
                    if kb == NKB - 1:
                        do_norm(h, qb, 0, 4)

            with band(fill_cur):
                proj_kq_fused(0, 0)
            for s in range(4):
                emit_scores(s)
            with band(fill_cur):
                proj_v(0, 0)
            pend = []
            for s in range(NS):
                pend.append(s)
                if pair_start(s):
                    emit_fill(s)
                    continue
                evs = emit_exp(pend)
                for t, ev in zip(pend, evs):
                    emit_pv(t, ev)
                # score region t%4 is free once the exp that read it has
                # been emitted; emission order defines the WAR dep
                for t in pend:
                    if t + 4 < NS:
                        emit_scores(t + 4)
                pend = []
                emit_fill(s)

    nc.compile()
    return nc


def _get_nc():
    if "nc" not in _CACHE:
        _CACHE["nc"] = _build()
    return _CACHE["nc"]


def _in_maps(x, Wq, bq, Wk, bk, Wv, bv):
    import ml_dtypes

    bf = ml_dtypes.bfloat16
    x = np.asarray(x, np.float32)
    maps = []
    for c in range(NCORES):
        b, hh = c // 2, c % 2
        cs = slice(hh * COLS, (hh + 1) * COLS)

        def warr(W):
            # [1024, 512] -> [128 p, 4 m, 8 j, 128 c]
            a = np.asarray(W, np.float32)[:, cs].astype(bf)
            return np.ascontiguousarray(
                a.reshape(8, 128, 4, 128).transpose(1, 2, 0, 3)
            )

        xTr = x[b].T.astype(bf).reshape(8, 128, 4, 512).transpose(1, 2, 0, 3)
        # wv: [1024, 512] -> [4 m, 128 p, 8 j * 128 c]
        wvr = np.asarray(Wv, np.float32)[:, cs].astype(bf)
        wvr = wvr.reshape(8, 128, 4, 128).transpose(2, 1, 0, 3).reshape(4, 128, 1024)
        maps.append(
            {
                "xT": np.ascontiguousarray(xTr),
                "wq": warr(Wq),
                "wk": warr(Wk),
                "wv": np.ascontiguousarray(wvr),
                "bqk": np.ascontiguousarray(
                    np.concatenate(
                        [
                            np.asarray(bq, np.float32)[cs].reshape(4, 128).T,
                            np.asarray(bk, np.float32)[cs].reshape(4, 128).T,
                        ],
                        axis=1,
                    )
                ),
                "bv": np.ascontiguousarray(np.asarray(bv, np.float32)[cs]),
            }
        )
    return maps


def _run(inputs, trace=False):
    from concourse import bass_utils

    nc = _get_nc()
    res = bass_utils.run_bass_kernel_spmd(
        nc,
        _in_maps(**inputs),
        core_ids=list(range(NCORES)),
        trace=trace,
    )
    out = np.empty((B, S, D), np.float32)
    for c in range(NCORES):
        b, hh = c // 2, c % 2
        out[b, :, hh * COLS : (hh + 1) * COLS] = res.results[c]["out"]
    return out, res


def kernel(**inputs):
    out, _ = _run(inputs, trace=False)
    return out


if __name__ == "__main__":
    _get_nc()
    print("build ok")


# revision 69
# speedup vs baseline: 1.0147x; 1.0010x over previous
"""Trainium2 Bass kernel for nn_AttentionLayer (B=4, S=2048, H=16, DH=64).

Sharding: 8 cores = 4 batches x 2 head-halves. Core c handles batch c//2,
heads (c%2)*8 .. (c%2)*8+8 (512 of the 1024 QKV columns).

Design (v2, all three engines balanced at ~195us busy; measured 1.12e-2
rel err on HW, modeled ~237us vs the 287us bf16/ACT-only baseline):
  - Projections (PE, bf16): x[2048,1024] @ W[1024,512] per head-pair chunk.
    Q/K write TRANSPOSED via DVE epilogues straight from PSUM into fp8.
  - Scores in fp8e4m3 with DoubleRow (0.5 cycles/row, halves score PE time):
    kt8[m] [128p=2 heads x 64dh, S] fp8 holds k8; qt8[m] [128, 2, S] holds
    (q8, qres8 = q - q8). The DoubleRow matmul contracts
    k8^T q8 + k8^T qres8 = k8^T q  -- the q-side fp8 quantization error is
    residual-corrected for free using the second k-tile slot (the k8 operand
    is a stride-0 broadcast AP, verified on HW). Only the k-side fp8 error
    (~2.5% elementwise) survives -> ~1% on probs.
  - exp is split between ACT (~62%) and DVE (~38%): ACT slots run
    activation(Exp, bf16 out); DVE slots run ONE tensor_scalar
    t = s*A + B with int16 output, A = 128*log2(e)/8,
    B = 16256 - 128*0.0573. The round-to-nearest convert makes the int16
    bit pattern EXACTLY the Schraudolph bf16 approximation of exp(s/8)
    (verified bit-exact on HW); PV reads the tile bitcast as bf16.
    ~1.7% RMS elementwise on the offloaded fraction only.
  - Score tiles are PER-SLOT ([128,512], tag sc, bufs=4 = 4 banks): four
    independent exp pipeline chains, so the sc-buffer round trip
    (exp -> score matmul -> exp, ~500ns) stays off the critical path on
    both exp engines.
  - ctx accumulates per (head, qb) group in PSUM (tag ctx, bufs=2), zeroed
    by a PE matmul of zeros (start=True stop=True): a clean accumulation
    group enforces the WAR dependency on the previous group's norm reads
    (plain start=True on the skip_group_check PVs races it -> NaN), and it
    keeps zeroing off the DVE critical path. V tiles carry a ones-column so
    PV accumulates the softmax denominator in ctx col 64; norm = DVE
    reciprocal + broadcast mul, one DMA per group (split in halves for the
    final group to shorten the tail).
  - PSUM banks: qkv 2 + sc 4 + ctx 2 = 8.
  - Group order: pairs 0,1 h-major; pairs 2/3 interleaved at group level so
    the exp-heavy tail is shared by two pairs, with pair-3 ramping in late
    enough that its K/Q/V projections run on the merged phase's PE slack.
    K/Q units and V quads are explicitly scheduled at slot positions
    (kq_sched) so the qkv psum rotation matches execution order.
  - Startup: DMA order wk0,wq0,x0(4),wv0,x1(2),x2(2),wv1,x3(2),wv2,wv3,
    wk1,wq1 (wv in per-pair chunks from a [4,128,1024] host layout). ~13
    throwaway fp32 matmuls ramp the PE p-state during the DMA window; the
    (m0,c0) K/Q projections chase x0 sub-chunk arrivals j-interleaved.
"""

import numpy as np

B, S, H, DH = 4, 2048, 16, 64
D = H * DH  # 1024
NCORES = 8
COLS = 512  # qkv columns per core (8 heads)
NKB = 16
EXP_SCALE = 0.125  # 1/sqrt(DH)
LOG2E = 1.4426950408889634
A_EXP = 128.0 * LOG2E * 0.125
B_EXP = 16256.0 - 128.0 * 0.0573  # bf16 exp bias + Schraudolph centering

_CACHE = {}


def _build():
    import concourse.mybir as mybir
    import concourse.tile as tile
    from concourse import bacc

    f32 = mybir.dt.float32
    bf16 = mybir.dt.bfloat16
    fp8 = mybir.dt.float8e4
    i16 = mybir.dt.int16
    Alu = mybir.AluOpType
    Exp = mybir.ActivationFunctionType.Exp
    DR = mybir.MatmulPerfMode.DoubleRow

    nc = bacc.Bacc(
        "TRN2",
        target_bir_lowering=False,
        debug=False,
        enable_asserts=False,
        num_devices=NCORES,
    )

    xT_d = nc.dram_tensor("xT", [128, 4, 8, 512], bf16, kind="ExternalInput").ap()
    wq_d = nc.dram_tensor("wq", [128, 4, 8, 128], bf16, kind="ExternalInput").ap()
    wk_d = nc.dram_tensor("wk", [128, 4, 8, 128], bf16, kind="ExternalInput").ap()
    wv_d = nc.dram_tensor("wv", [4, 128, 1024], bf16, kind="ExternalInput").ap()
    bqk_d = nc.dram_tensor("bqk", [128, 8], f32, kind="ExternalInput").ap()
    bv_d = nc.dram_tensor("bv", [COLS], f32, kind="ExternalInput").ap()
    out_d = nc.dram_tensor("out", [S, COLS], f32, kind="ExternalOutput").ap()

    with tile.TileContext(nc) as tc:
        with (
            tc.tile_pool(name="consts", bufs=1) as consts,
            tc.tile_pool(name="wpool", bufs=1) as wpool,
            tc.tile_pool(name="qkp", bufs=1) as qkp,
            tc.tile_pool(name="vpool", bufs=1) as vpool,
            tc.tile_pool(name="xpool", bufs=1) as xpool,
            tc.tile_pool(name="epool", bufs=1) as epool,
            tc.tile_pool(name="opool", bufs=1) as opool,
            tc.tile_pool(name="psum", bufs=1, space="PSUM") as psum,
        ):
            from contextlib import contextmanager

            base = tc.cur_priority + 50
            att_cur = [base]
            fill_cur = [base + 8000]

            @contextmanager
            def band(cursor):
                off = tc.cur_priority - cursor[0]
                with tc.high_priority(offset=off):
                    yield
                    cursor[0] = tc.cur_priority

            # ---- constants + PE p-state warm-up ----
            with band(att_cur):
                warm = consts.tile([1, 1], f32)
                nc.vector.memset(warm, 0.0)
                nc.scalar.activation(warm, warm, Exp)  # pull ACT table load
                wsrc = consts.tile([128, 128], f32, name="wsrc")
                nc.vector.memset(wsrc, 0.0)
                zsrc = consts.tile([128, 260], mybir.dt.bfloat16, name="zsrc")
                nc.vector.memset(zsrc, 0.0)
                # single persistent 4-bank score tile; slots rotate regions
                # manually (subtile deps track the RAW/WAR ordering). ACT
                # exp reads region PAIRS [0:2] in one instruction.
                sc_all = psum.tile([128, 4, 512], f32, tag="sc", bufs=1,
                                   name="sc_all")
                for i in range(13):
                    nc.tensor.matmul(
                        sc_all[:, i % 4, 0:64],
                        lhsT=wsrc,
                        rhs=wsrc[:, 0:64],
                        start=True,
                        stop=True,
                    )

            with band(fill_cur):
                bqk_t = consts.tile([128, 8], f32)
                bv_s = consts.tile([1, COLS], f32)
                bvb = consts.tile([128, COLS], f32)
                nc.gpsimd.dma_start(out=bqk_t, in_=bqk_d)
                bq_t = bqk_t[:, 0:4]
                bk_t = bqk_t[:, 4:8]
                nc.gpsimd.dma_start(out=bv_s, in_=bv_d[None, :])
                nc.gpsimd.partition_broadcast(bvb, bv_s)

                vt = vpool.tile([128, NKB, 8, 65], bf16, name="vt")
                nc.vector.memset(vt[:, :, :, 64:65], 1.0)

                # fp8 q/k tiles: kt8[m] [128p = 2 heads x 64 dh, S];
                # qt8[m] [128, 2, S] = (q8, qres8)
                kt8 = [qkp.tile([128, S], fp8, name=f"kt8{m}") for m in range(4)]
                qt8 = [qkp.tile([128, 2, S], fp8, name=f"qt8{m}") for m in range(4)]
                wvt = [wpool.tile([128, 8, 128], bf16, name=f"wvt{m}") for m in range(4)]

            # ---- weight chunk ring ----
            wcur = {"q": {}, "k": {}}

            def load_w(proj, m):
                w_d = wq_d if proj == "q" else wk_d
                wt = wpool.tile(
                    [128, 8, 128], bf16, tag=f"w{proj}", bufs=2, name=f"w{proj}{m}"
                )
                nc.sync.dma_start(out=wt, in_=w_d[:, m, :, :])
                wcur[proj][m] = wt

            def load_wv(m):
                nc.sync.dma_start(
                    out=wvt[m], in_=wv_d[m].rearrange("p (j c) -> p j c", j=8)
                )

            with band(fill_cur):
                xt = [
                    xpool.tile([128, 8, 512], bf16, name=f"xt{c}") for c in range(4)
                ]

                # DMA order: startup-critical first; x0 split fine for
                # matmul chasing, later chunks coarser.
                load_w("k", 0)
                load_w("q", 0)
                for j0 in range(0, 8, 2):
                    nc.sync.dma_start(
                        out=xt[0][:, j0 : j0 + 2, :], in_=xT_d[:, 0, j0 : j0 + 2, :]
                    )
                load_wv(0)
                for j0 in range(0, 8, 4):
                    nc.sync.dma_start(
                        out=xt[1][:, j0 : j0 + 4, :], in_=xT_d[:, 1, j0 : j0 + 4, :]
                    )
                for j0 in range(0, 8, 4):
                    nc.sync.dma_start(
                        out=xt[2][:, j0 : j0 + 4, :], in_=xT_d[:, 2, j0 : j0 + 4, :]
                    )
                for j0 in range(0, 8, 4):
                    nc.sync.dma_start(
                        out=xt[3][:, j0 : j0 + 4, :], in_=xT_d[:, 3, j0 : j0 + 4, :]
                    )
                load_wv(1)
                load_wv(2)
                load_wv(3)
                load_w("k", 1)
                load_w("q", 1)

            # ---- projection epilogues (DVE, PSUM -> fp8 SBUF) ----
            def epi_k(m, c, ps):
                ch = slice(c * 512, (c + 1) * 512)
                with band(att_cur):
                    nc.vector.tensor_scalar_add(
                        kt8[m][:, ch], ps, bk_t[:, m : m + 1])

            def epi_q(m, c, ps):
                ch = slice(c * 512, (c + 1) * 512)
                with band(att_cur):
                    nc.vector.tensor_scalar_add(
                        qt8[m][:, 0, ch], ps, bq_t[:, m : m + 1])
                    nc.vector.scalar_tensor_tensor(
                        qt8[m][:, 1, ch],
                        ps,
                        bq_t[:, m : m + 1],
                        qt8[m][:, 0, ch],
                        Alu.add,
                        Alu.subtract,
                    )

            # ---- projection unit emitters (fill band) ----
            # Units are split into sub-chain parts emitted a couple of slots
            # apart so long PE chains do not clump ahead of the score
            # matmuls in PE program order.
            proj_state = {}

            def proj_kq_fused(m, c, part=None):
                key = ("kq", m, c)
                parts = range(4) if part is None else [part]
                for p in parts:
                    if p == 0:
                        proj_state[key] = (
                            psum.tile([128, 512], f32, tag="qkv", bufs=2,
                                      name="psk"),
                            psum.tile([128, 512], f32, tag="qkv", bufs=2,
                                      name="psq2"),
                        )
                    psk, psq = proj_state[key]
                    for j in (2 * p, 2 * p + 1):
                        nc.tensor.matmul(
                            psk, lhsT=wcur["k"][m][:, j, :], rhs=xt[c][:, j, :],
                            start=(j == 0), stop=(j == 7),
                        )
                        nc.tensor.matmul(
                            psq, lhsT=wcur["q"][m][:, j, :], rhs=xt[c][:, j, :],
                            start=(j == 0), stop=(j == 7),
                        )
                    if p == 3:
                        epi_k(m, c, psk)
                        epi_q(m, c, psq)

            def proj_qk(proj, m, c, part):
                key = (proj, m, c)
                if part == 0:
                    proj_state[key] = psum.tile(
                        [128, 512], f32, tag="qkv", bufs=2, name="psq"
                    )
                ps = proj_state[key]
                w = wcur[proj][m]
                for j in (range(4) if part == 0 else range(4, 8)):
                    nc.tensor.matmul(
                        ps,
                        lhsT=w[:, j, :],
                        rhs=xt[c][:, j, :],
                        start=(j == 0),
                        stop=(j == 7),
                    )
                if part == 1:
                    (epi_q if proj == "q" else epi_k)(m, c, ps)

            def proj_v(m, c, part=None):
                # 4 seq-subchunks of chunk c in one psum tile + one batched
                # DVE add into the vt tile; one sub-chain per part
                key = ("v", m, c)
                parts = range(4) if part is None else [part]
                for i in parts:
                    if i == 0:
                        proj_state[key] = psum.tile(
                            [128, 512], f32, tag="qkv", bufs=2, name="psv"
                        )
                    ps = proj_state[key]
                    for j in range(8):
                        nc.tensor.matmul(
                            ps[:, i * 128 : (i + 1) * 128],
                            lhsT=xt[c][:, j, i * 128 : (i + 1) * 128],
                            rhs=wvt[m][:, j, :],
                            # one start per bank-life: later sub-chunks'
                            # first writes consume the pending-zero bytes
                            start=(i == 0 and j == 0),
                            stop=(i == 3 and j == 7),
                            skip_group_check=True,
                        )
                    if i == 3:
                        with band(att_cur):
                            emit_vadd(m, c, ps)

            def emit_vadd(m, c, ps):
                        nc.vector.tensor_add(
                            vt[:, 4 * c : 4 * c + 4, 2 * m : 2 * m + 2, 0:64],
                            ps.rearrange("p (i h d) -> p i h d", i=4, h=2),
                            bvb[:, m * 128 : (m + 1) * 128]
                            .rearrange("p (one h d) -> p one h d", one=1, h=2)
                            .broadcast_to([128, 4, 2, 64]),
                        )

            # ---- attention stream ----
            # pair-0/1 groups h-major; pairs 2 and 3 interleave at group
            # granularity so the exp-heavy tail is shared by both pairs
            # (pair-3 slots start right after its projections land). The
            # first two groups (h0/h1 at qb0) interleave per k-block (each
            # gets its own ctx bank via bufs=2).
            groups = [(h, qb) for h in (0, 1, 2, 3) for qb in range(4)]
            # pair-2 first, then pairs 2/3 interleaved: pair-3's first group
            # starts at slot 336, so ALL of pair-3's K/Q/V projection work
            # lands in the merged phase's PE-slack zone (t>~150)
            groups += [(4, 0), (4, 1), (4, 2), (4, 3), (5, 0), (6, 0),
                       (5, 1), (6, 1), (5, 2), (6, 2), (5, 3), (6, 3),
                       (7, 0), (7, 1), (7, 2), (7, 3)]
            slots = []
            for kb in range(NKB):
                slots.append((0, 0, kb))
                slots.append((1, 0, kb))
            slots += [
                (h, qb, kb)
                for (h, qb) in groups
                if (h, qb) not in ((0, 0), (1, 0))
                for kb in range(NKB)
            ]
            NS = len(slots)

            # engine assignment: 3/8 DVE in the proj-heavy first half,
            # 7/16 in the balanced second half; none in the startup slots
            # (DVE is busy with the critical projection epilogues there).
            # DVE slots sit at s%4 in {2,3} so that ACT slots form
            # region-aligned pairs [0:2] sharing one exp instruction.
            def is_dve(s):
                if s < 24:
                    return False
                if s >= NS - 3:
                    return s % 2 == 1
                if s < 256:
                    return s % 8 in (2, 3, 7)
                return s % 16 in (2, 3, 6, 7, 10, 11, 14)

            def pair_start(s):
                return (
                    s % 4 == 0
                    and s + 1 < NS
                    and not is_dve(s)
                    and not is_dve(s + 1)
                )

            # All projection units explicitly scheduled at slot positions so
            # the qkv psum rotation matches execution order. V quads for
            # pair m land shortly before pair-m attention consumes them.
            kq_sched = {}

            def sched(s, item):
                kq_sched.setdefault(s, []).append(item)

            PSPC = 0  # sub-chain parts of one unit are emitted together

            def sched_qk(s, proj, m, c):
                sched(s, (proj, m, c, 0))
                sched(s + PSPC, (proj, m, c, 1))

            def sched_v(s, m, c):
                for i in range(4):
                    sched(s + PSPC * i, ("v", m, c, i))

            # pair 0: k chunks feed the kb-consuming prefix (early); q chunks
            # are only needed per qb-group start (32/48/64) so they spread
            for c in range(1, 4):
                sched_qk(4 * c - 2, "k", 0, c)
                sched_qk(8 + 16 * (c - 1), "q", 0, c)
            for c in range(1, 4):  # v(0,0) is emitted in the prologue
                sched_v(8 * c - 6, 0, c)

            def sched_pair(m, kq0, ks, qs, vs):
                if m >= 2:  # pair-1 weights are loaded in the startup block
                    sched(kq0 - 6, ("wl", "k", m))
                    sched(kq0 - 4, ("wl", "q", m))
                for p in range(4):
                    sched(kq0 + PSPC * p, ("kq", m, 0, p))
                for c in range(1, 4):
                    sched_qk(ks[c - 1], "k", m, c)
                    sched_qk(qs[c - 1], "q", m, c)
                for c in range(4):
                    sched_v(vs + 4 * c, m, c)

            # pair 1 window starts slot 128; pair 2 at 256; pair 3 at 336
            sched_pair(1, 66, (72, 82, 92), (76, 86, 96), 104)
            sched_pair(2, 136, (146, 156, 166), (248, 268, 288), 224)
            sched_pair(3, 288, (296, 304, 312), (326, 356, 386), 316)

            # pair-3's projection units are emitted in the ATTENTION band:
            # a hard position in PE program order prevents the greedy
            # scheduler from deferring them until their consumers stall;
            # earlier pairs stay in the fill band (their windows have real
            # PE contention where floating helps)
            def emit_fill(s):
                for u in kq_sched.get(s, ()):
                    m = u[2] if u[0] in ("wl",) else u[1]
                    with band(att_cur if m == 3 else fill_cur):
                        if u[0] == "wl":
                            load_w(u[1], u[2])
                        elif u[0] == "kq":
                            proj_kq_fused(u[1], u[2], u[3])
                        elif u[0] == "v":
                            proj_v(u[1], u[2], u[3])
                        else:
                            proj_qk(u[0], u[1], u[2], u[3])

            def emit_scores(s):
                h, qb, kb = slots[s]
                m, p0 = h // 2, 64 * (h % 2)
                with band(att_cur):
                    kb_ap = (
                        kt8[m][p0 : p0 + 64, kb * 128 : (kb + 1) * 128]
                        .rearrange("p (one f) -> p one f", one=1)
                        .broadcast_to([64, 2, 128])
                    )
                    nc.tensor.matmul(
                        sc_all[:, s % 4, :],
                        lhsT=kb_ap,
                        rhs=qt8[m][p0 : p0 + 64, :, qb * 512 : (qb + 1) * 512],
                        start=True,
                        stop=True,
                        perf_mode=DR,
                    )

            def emit_exp(pend):
                # pend: 1 slot (single/DVE) or 2 ACT slots on regions [0:2]
                s0 = pend[0]
                with band(att_cur):
                    if len(pend) == 2:
                        ee = epool.tile([128, 2, 512], mybir.dt.bfloat16,
                                        tag="ep", bufs=4, name="ep")
                        nc.scalar.activation(
                            ee, sc_all[:, 0:2, :], Exp, scale=EXP_SCALE
                        )
                        return [ee[:, 0, :], ee[:, 1, :]]
                    if is_dve(s0):
                        ed = epool.tile([128, 512], i16, tag="ed", bufs=6,
                                        name="ed")
                        nc.vector.tensor_scalar(
                            ed, sc_all[:, s0 % 4, :], A_EXP, B_EXP,
                            Alu.mult, Alu.add,
                        )
                        return [ed.bitcast(mybir.dt.bfloat16)]
                    ee = epool.tile([128, 512], mybir.dt.bfloat16, tag="ee",
                                    bufs=6, name="ee")
                    nc.scalar.activation(
                        ee, sc_all[:, s0 % 4, :], Exp, scale=EXP_SCALE
                    )
                    return [ee]

            ctx_of = {}

            def do_pv(ev, h, qb, kb):
                if kb == 0:
                    ctx_of[(h, qb)] = psum.tile(
                        [128, 4, 65], f32, tag="ctx", bufs=2, name="ctx"
                    )
                    # zero via a PE matmul: a clean start/stop group, so the
                    # WAR dep on the previous group's norm reads is enforced
                    # (skip_group_check PVs alone race it), and it keeps the
                    # zeroing off the DVE critical path.
                    nc.tensor.matmul(
                        ctx_of[(h, qb)].rearrange("p t d -> p (t d)"),
                        lhsT=zsrc[:, 0:128],
                        rhs=zsrc,
                        start=True,
                        stop=True,
                    )
                ctx = ctx_of[(h, qb)]
                for qt in range(4):
                    nc.tensor.matmul(
                        ctx[:, qt, :],
                        lhsT=ev[:, qt * 128 : (qt + 1) * 128],
                        rhs=vt[:, kb, h, :],
                        start=False,
                        stop=False,
                        skip_group_check=True,
                    )

            def do_norm(h, qb, t0, t1):
                ctx = ctx_of[(h, qb)]
                nt = t1 - t0
                rr = opool.tile([128, 4, 1], f32, tag="r", bufs=3, name="rr")
                nc.vector.reciprocal(rr[:, t0:t1], ctx[:, t0:t1, 64:65])
                ob = opool.tile([128, 4, 64], f32, tag="o", bufs=3, name="ob")
                nc.vector.tensor_mul(
                    ob[:, t0:t1], ctx[:, t0:t1, 0:64],
                    rr[:, t0:t1].broadcast_to([128, nt, 64]),
                )
                nc.sync.dma_start(
                    out=out_d[
                        qb * 512 + t0 * 128 : qb * 512 + t1 * 128,
                        h * 64 : (h + 1) * 64,
                    ].rearrange("(t p) d -> p t d", p=128),
                    in_=ob[:, t0:t1],
                )

            last_group = groups[-1]

            def emit_pv(s, ev):
                h, qb, kb = slots[s]
                with band(att_cur):
                    if kb == NKB - 1 and (h, qb) == last_group:
                        # split the final norm to shorten the tail
                        ctx = ctx_of[(h, qb)]
                        for qt in range(4):
                            nc.tensor.matmul(
                                ctx[:, qt, :],
                                lhsT=ev[:, qt * 128 : (qt + 1) * 128],
                                rhs=vt[:, kb, h, :],
                                start=False,
                                stop=False,
                                skip_group_check=True,
                            )
                            if qt == 1:
                                do_norm(h, qb, 0, 2)
                        do_norm(h, qb, 2, 4)
                        return
                    do_pv(ev, h, qb, kb)
                    if kb == NKB - 1:
                        do_norm(h, qb, 0, 4)

            with band(fill_cur):
                proj_kq_fused(0, 0)
            for s in range(4):
                emit_scores(s)
            with band(fill_cur):
                proj_v(0, 0)
            pend = []
            for s in range(NS):
                pend.append(s)
                if pair_start(s):
                    emit_fill(s)
                    continue
                evs = emit_exp(pend)
                for t, ev in zip(pend, evs):
                    emit_pv(t, ev)
                # score region t%4 is free once the exp that read it has
                # been emitted; emission order defines the WAR dep
                for t in pend:
                    if t + 4 < NS:
                        emit_scores(t + 4)
                pend = []
                emit_fill(s)

    nc.compile()
    return nc


def _get_nc():
    if "nc" not in _CACHE:
        _CACHE["nc"] = _build()
    return _CACHE["nc"]


def _in_maps(x, Wq, bq, Wk, bk, Wv, bv):
    import ml_dtypes

    bf = ml_dtypes.bfloat16
    x = np.asarray(x, np.float32)
    maps = []
    for c in range(NCORES):
        b, hh = c // 2, c % 2
        cs = slice(hh * COLS, (hh + 1) * COLS)

        def warr(W):
            # [1024, 512] -> [128 p, 4 m, 8 j, 128 c]
            a = np.asarray(W, np.float32)[:, cs].astype(bf)
            return np.ascontiguousarray(
                a.reshape(8, 128, 4, 128).transpose(1, 2, 0, 3)
            )

        xTr = x[b].T.astype(bf).reshape(8, 128, 4, 512).transpose(1, 2, 0, 3)
        # wv: [1024, 512] -> [4 m, 128 p, 8 j * 128 c]
        wvr = np.asarray(Wv, np.float32)[:, cs].astype(bf)
        wvr = wvr.reshape(8, 128, 4, 128).transpose(2, 1, 0, 3).reshape(4, 128, 1024)
        maps.append(
            {
                "xT": np.ascontiguousarray(xTr),
                "wq": warr(Wq),
                "wk": warr(Wk),
                "wv": np.ascontiguousarray(wvr),
                "bqk": np.ascontiguousarray(
                    np.concatenate(
                        [
                            np.asarray(bq, np.float32)[cs].reshape(4, 128).T,
                            np.asarray(bk, np.float32)[cs].reshape(4, 128).T,
                        ],
                        axis=1,
                    )
                ),
                "bv": np.ascontiguousarray(np.asarray(bv, np.float32)[cs]),
            }
        )
    return maps


def _run(inputs, trace=False):
    from concourse import bass_utils

    nc = _get_nc()
    res = bass_utils.run_bass_kernel_spmd(
        nc,
        _in_maps(**inputs),
        core_ids=list(range(NCORES)),
        trace=trace,
    )
    out = np.empty((B, S, D), np.float32)
    for c in range(NCORES):
        b, hh = c // 2, c % 2
        out[b, :, hh * COLS : (hh + 1) * COLS] = res.results[c]["out"]
    return out, res


def kernel(**inputs):
    out, _ = _run(inputs, trace=False)
    return out


if __name__ == "__main__":
    _get_nc()
    print("build ok")
